# revision 4
# baseline (speedup 1.0000x reference)
"""ChromosomeEmbedding kernel for 8x Trainium2 NeuronCores.

Computes out[b, j, d] = ce[chr[b]-1, d] for b in [0,512), j in [0,2001),
d in [0,128). Data-parallel: the batch is sharded 64 samples/core across
8 cores; the 24x128 table lookup (64 rows -> 32 KB) is folded into host
input prep, so the device program is a pure HBM-write streamer. The
per-core output shard is 65.5 MB.

Key HW fact (measured via per-descriptor NTFF records): a 64-partition
dma_start spreads its descriptors across all 16 SDMA engines, so engines
read SBUF partitions that live on OTHER engines' AXI ports. The port
collisions cap each engine at ~18-19 GB/s (~286 GB/s/core aggregate)
even though every engine is 99% busy. A 128-partition DMA gets the
canonical descriptor swizzle -- engine k reads only the 8 partitions on
port k -- which removes the collisions.

So every output DMA here is a 128-partition "dual-block" transfer:
SBUF partitions 0:64 hold the 64 sample rows, partitions 64:128 hold an
identical copy (both halves replicated to REP=64 bin-columns), and one
DMA writes bins [a, a+w) from the low half plus bins [a+w, a+2w) from
the high half via a (2, 64, w, 128) DRAM-side access pattern.

Per-core device program (identical SPMD program on all cores, raw bacc):
  1. Sync ring loads replica seed columns 0:4, scalar ring columns 4:8
     (both 128-partition), so each ring can open its ladder after only
     its own 256 KB load.
  2. Three doubling copies on the vector engine extend 8 -> 64 columns.
  3. Each ring opens with a ladder (w = 4, 8, 16, 32, 64 -> 8..128 bins
     per DMA, gated on the replica width available) then streams 128-bin
     4 MB DMAs. Sync walks bins [0, SPLIT), scalar [SPLIT, 2001); the
     final remainder DMA overlaps one already-written bin when the range
     is odd (identical bytes, so order doesn't matter).
  4. Minimal tail: each ring waits for its completion count, bumps a
     done-sem; gpsimd resets DMA state and clears sems for re-execution.
"""

import functools

import numpy as np

from concourse import bacc, mybir
from concourse.bass_utils import run_bass_kernel_spmd

N_CORES = 8
BS = 512
BPC = BS // N_CORES  # 64 samples per core
NBIN = 2001
DIM = 128
N_CHR = 24
REP = 64  # replicated copies of each row held in SBUF (per partition half)
W0 = 8  # host-side pre-replication width (bins) in the input tensor
SPLIT = 1000  # bins walked by the sync ring; scalar ring takes the rest
F32 = mybir.dt.float32


def _need_v(w):
    """Doubling-copy count required before rep[:, 0:w] is valid."""
    if w <= W0:
        return 0
    v = 0
    have = W0
    while have < w:
        have *= 2
        v += 1
    return v


def _ring_plan(lo, hi, seed_lo):
    """Cover bins [lo, hi) with dual-block DMAs: list of (out_off, src_off,
    w, v) where the DMA writes [out_off, out_off+2w) sourcing rep[:,
    src_off:src_off+w]. First rung uses this ring's own seed columns
    [seed_lo, seed_lo+4) so it needs no cross-ring wait. If the range is
    odd, the final DMA overlaps one bin already covered (same bytes)."""
    n = hi - lo
    plan = []
    off = lo
    for w, src in [(4, seed_lo), (8, 0), (16, 0), (32, 0), (64, 0)]:
        if hi - off < 2 * w:
            break
        plan.append((off, src, w, _need_v(w + src)))
        off += 2 * w
    while hi - off >= 2 * REP:
        plan.append((off, 0, REP, 3))
        off += 2 * REP
    r = hi - off
    if r > 0:
        w = (r + 1) // 2  # covers 2w >= r bins, overlapping (2w - r) bins
        plan.append((hi - 2 * w, 0, w, _need_v(w)))
    return plan


@functools.lru_cache(maxsize=1)
def build_nc():
    nc = bacc.Bacc("TRN2", target_bir_lowering=False)

    pre_h = nc.declare_dram_parameter("pre", [128, W0, DIM], F32, isOutput=False)
    out_h = nc.declare_dram_parameter("out", [BPC, NBIN, DIM], F32, isOutput=True)

    with (
        nc.sbuf_tensor("rep", [128, REP, DIM], F32) as rep,
        nc.semaphore("ssem") as ssem,  # sync-ring DMA completions
        nc.semaphore("asem") as asem,  # scalar-ring DMA completions
        nc.semaphore("vsem") as vsem,  # doubling-copy completions
        nc.semaphore("done") as done,  # ring-drained markers
    ):
        sync_plan = _ring_plan(0, SPLIT, 0)
        scal_plan = _ring_plan(SPLIT, NBIN, 4)

        # Each ring loads its own seed columns (128-partition loads).
        nc.sync.dma_start(out=rep[:, 0:4, :], in_=pre_h[:, 0:4, :]).then_inc(ssem, 16)
        nc.scalar.dma_start(out=rep[:, 4:8, :], in_=pre_h[:, 4:8, :]).then_inc(
            asem, 16
        )

        # Vector engine: doubling replication W0 -> REP columns (needs both
        # seed halves loaded).
        nc.vector.wait_ge(ssem, 16)
        nc.vector.wait_ge(asem, 16)
        w = W0
        while w < REP:
            nc.vector.tensor_copy(
                out=rep[:, w : 2 * w, :], in_=rep[:, 0:w, :]
            ).then_inc(vsem, 1)
            w *= 2

        def dual_out(off, w):
            """(2, 64, w, DIM) view of out_h[:, off:off+2w, :]: partition
            p = r*64 + b writes sample b, bins [off+r*w, off+(r+1)*w)."""
            return (
                out_h[:, off : off + 2 * w, :]
                .rearrange("b (r w) d -> b r w d", r=2)
                .transpose([1, 0, 2, 3])
            )

        def run_ring(eng, plan, own_sem, own_ready, other_sem):
            eng.wait_ge(own_sem, own_ready)
            seen_v = 0
            for i, (off, src, w, v) in enumerate(plan):
                if i == 1:
                    # Rung 0 reads only this ring's own seed columns;
                    # every later rung reads rep[:, 0:w] (w >= 8), which
                    # needs the other ring's seed columns too.
                    eng.wait_ge(other_sem, 16)
                if v > seen_v:
                    eng.wait_ge(vsem, v)
                    seen_v = v
                eng.dma_start(
                    out=dual_out(off, w), in_=rep[:, src : src + w, :]
                ).then_inc(own_sem, 16)

        run_ring(nc.sync, sync_plan, ssem, 16, asem)
        run_ring(nc.scalar, scal_plan, asem, 16, ssem)

        # Tail: wait for both rings to drain, then restore sem state so
        # the NEFF can be re-executed (sems are only load-time zeroed).
        nc.sync.wait_ge(ssem, 16 * (1 + len(sync_plan)))
        nc.sync.sem_inc(done, 1)
        nc.scalar.wait_ge(asem, 16 * (1 + len(scal_plan)))
        nc.scalar.sem_inc(done, 1)

        nc.gpsimd.wait_ge(done, 2)
        nums = sorted(s.num for s in (ssem, asem, vsem, done))
        lo, hi = nums[0], nums[-1]
        if nums == list(range(lo, hi + 1)):
            ranges = [range(lo, hi + 1)]
        else:
            ranges = [range(n, n + 1) for n in nums]
        for r in ranges:
            nc.gpsimd.dma_reset(r)
            nc.gpsimd.sem_clear(r)

    nc.compile()
    return nc


def make_in_maps(chr_full: np.ndarray, ce: np.ndarray):
    ce_f32 = np.asarray(ce, dtype=np.float32)
    idx = np.asarray(chr_full).astype(np.int64) - 1
    maps = []
    for c in range(N_CORES):
        rows = ce_f32[idx[c * BPC : (c + 1) * BPC]]  # [64, 128]
        both = np.concatenate([rows, rows], axis=0)  # [128, 128]
        pre = np.repeat(both[:, None, :], W0, axis=1)  # [128, W0, 128]
        maps.append({"pre": np.ascontiguousarray(pre)})
    return maps


def kernel(tensor=None, chr=None, ce=None, **_unused):
    chr_np = np.asarray(chr)
    ce_np = np.asarray(ce)
    nc = build_nc()
    res = run_bass_kernel_spmd(
        nc, make_in_maps(chr_np, ce_np), core_ids=list(range(N_CORES))
    )
    out = np.concatenate([r["out"] for r in res.results], axis=0)
    return out.astype(np.float32)


# revision 7
# speedup vs baseline: 6.7937x; 6.7937x over previous
"""ChromosomeEmbedding kernel for 8x Trainium2 NeuronCores.

Computes out[b, j, d] = ce[chr[b]-1, d] for b in [0,512), j in [0,2001),
d in [0,128). Data-parallel: the batch is sharded 64 samples/core across
8 cores; the 24x128 table lookup (64 rows -> 32 KB) is folded into host
input prep, so the device program is a pure HBM-write streamer. The
per-core output shard is 65.5 MB.

Key HW fact (measured via per-descriptor NTFF records): a 64-partition
dma_start spreads its descriptors across all 16 SDMA engines, so engines
read SBUF partitions that live on OTHER engines' AXI ports. The port
collisions cap each engine at ~18-19 GB/s (~286 GB/s/core aggregate)
even though every engine is 99% busy. A 128-partition DMA gets the
canonical descriptor swizzle -- engine k reads only the 8 partitions on
port k -- which removes the collisions.

So every output DMA here is a 128-partition "dual-block" transfer:
SBUF partition p holds sample p//2 (each sample duplicated on an
adjacent partition pair, replicated to REP=64 bin-columns), and one DMA
writes bins [a, a+w) from even-offset copies plus bins [a+w, a+2w) from
odd-offset copies via a (64, 2, w, 128) DRAM-side access pattern whose
outer dims walk partitions 0..127 in order -- keeping the DRAM outer
extent at 128 is required for the descriptor-to-engine spread.

Per-core device program (identical SPMD program on all cores, raw bacc):
  1. Sync ring loads replica seed columns 0:4, scalar ring columns 4:8
     (both 128-partition), so each ring can open its ladder after only
     its own 256 KB load.
  2. Three doubling copies on the vector engine extend 8 -> 64 columns.
  3. Each ring opens with a ladder (w = 4, 8, 16, 32, 64 -> 8..128 bins
     per DMA, gated on the replica width available) then streams 128-bin
     4 MB DMAs. Sync walks bins [0, SPLIT), scalar [SPLIT, 2001); the
     final remainder DMA overlaps one already-written bin when the range
     is odd (identical bytes, so order doesn't matter).
  4. Minimal tail: each ring waits for its completion count, bumps a
     done-sem; gpsimd resets DMA state and clears sems for re-execution.
"""

import functools

import numpy as np

from concourse import bacc, mybir
from concourse.bass_utils import run_bass_kernel_spmd

N_CORES = 8
BS = 512
BPC = BS // N_CORES  # 64 samples per core
NBIN = 2001
DIM = 128
N_CHR = 24
REP = 64  # replicated copies of each row held in SBUF (per partition half)
W0 = 8  # host-side pre-replication width (bins) in the input tensor
SPLIT = 1000  # bins walked by the sync ring; scalar ring takes the rest
F32 = mybir.dt.float32


def _need_v(w):
    """Doubling-copy count required before rep[:, 0:w] is valid."""
    if w <= W0:
        return 0
    v = 0
    have = W0
    while have < w:
        have *= 2
        v += 1
    return v


def _ring_plan(lo, hi, seed_lo):
    """Cover bins [lo, hi) with dual-block DMAs: list of (out_off, src_off,
    w, v) where the DMA writes [out_off, out_off+2w) sourcing rep[:,
    src_off:src_off+w]. First rung uses this ring's own seed columns
    [seed_lo, seed_lo+4) so it needs no cross-ring wait. If the range is
    odd, the final DMA overlaps one bin already covered (same bytes)."""
    n = hi - lo
    plan = []
    off = lo
    for w, src in [(4, seed_lo), (8, 0), (16, 0), (32, 0), (64, 0)]:
        if hi - off < 2 * w:
            break
        plan.append((off, src, w, _need_v(w + src)))
        off += 2 * w
    while hi - off >= 2 * REP:
        plan.append((off, 0, REP, 3))
        off += 2 * REP
    r = hi - off
    if r > 0:
        w = (r + 1) // 2  # covers 2w >= r bins, overlapping (2w - r) bins
        plan.append((hi - 2 * w, 0, w, _need_v(w)))
    return plan


@functools.lru_cache(maxsize=1)
def build_nc():
    nc = bacc.Bacc("TRN2", target_bir_lowering=False)

    pre_h = nc.declare_dram_parameter("pre", [128, W0, DIM], F32, isOutput=False)
    out_h = nc.declare_dram_parameter("out", [BPC, NBIN, DIM], F32, isOutput=True)

    with (
        nc.sbuf_tensor("rep", [128, REP, DIM], F32) as rep,
        nc.semaphore("ssem") as ssem,  # sync-ring DMA completions
        nc.semaphore("asem") as asem,  # scalar-ring DMA completions
        nc.semaphore("vsem") as vsem,  # doubling-copy completions
        nc.semaphore("done") as done,  # ring-drained markers
    ):
        sync_plan = _ring_plan(0, SPLIT, 0)
        scal_plan = _ring_plan(SPLIT, NBIN, 4)

        # Each ring loads its own seed columns (128-partition loads).
        nc.sync.dma_start(out=rep[:, 0:4, :], in_=pre_h[:, 0:4, :]).then_inc(ssem, 16)
        nc.scalar.dma_start(out=rep[:, 4:8, :], in_=pre_h[:, 4:8, :]).then_inc(
            asem, 16
        )

        # Vector engine: doubling replication W0 -> REP columns (needs both
        # seed halves loaded).
        nc.vector.wait_ge(ssem, 16)
        nc.vector.wait_ge(asem, 16)
        w = W0
        while w < REP:
            nc.vector.tensor_copy(
                out=rep[:, w : 2 * w, :], in_=rep[:, 0:w, :]
            ).then_inc(vsem, 1)
            w *= 2

        def dual_out(off, w):
            """(64, 2, w, DIM) view of out_h[:, off:off+2w, :]: partition
            p = 2b + r writes sample b, bins [off+r*w, off+(r+1)*w).
            Keeping the DRAM-side outer dims (64, 2) = 128 aligned 1:1
            with SBUF partitions is what preserves the canonical
            descriptor-to-engine swizzle (a small outer dim collapses the
            whole DMA onto one or two SDMA engines)."""
            return out_h[:, off : off + 2 * w, :].rearrange(
                "b (r w) d -> b r w d", r=2
            )

        def run_ring(eng, plan, own_sem, own_ready, other_sem):
            eng.wait_ge(own_sem, own_ready)
            seen_v = 0
            for i, (off, src, w, v) in enumerate(plan):
                if i == 1:
                    # Rung 0 reads only this ring's own seed columns;
                    # every later rung reads rep[:, 0:w] (w >= 8), which
                    # needs the other ring's seed columns too.
                    eng.wait_ge(other_sem, 16)
                if v > seen_v:
                    eng.wait_ge(vsem, v)
                    seen_v = v
                eng.dma_start(
                    out=dual_out(off, w), in_=rep[:, src : src + w, :]
                ).then_inc(own_sem, 16)

        run_ring(nc.sync, sync_plan, ssem, 16, asem)
        run_ring(nc.scalar, scal_plan, asem, 16, ssem)

        # Tail: wait for both rings to drain, then restore sem state so
        # the NEFF can be re-executed (sems are only load-time zeroed).
        nc.sync.wait_ge(ssem, 16 * (1 + len(sync_plan)))
        nc.sync.sem_inc(done, 1)
        nc.scalar.wait_ge(asem, 16 * (1 + len(scal_plan)))
        nc.scalar.sem_inc(done, 1)

        nc.gpsimd.wait_ge(done, 2)
        nums = sorted(s.num for s in (ssem, asem, vsem, done))
        lo, hi = nums[0], nums[-1]
        if nums == list(range(lo, hi + 1)):
            ranges = [range(lo, hi + 1)]
        else:
            ranges = [range(n, n + 1) for n in nums]
        for r in ranges:
            nc.gpsimd.dma_reset(r)
            nc.gpsimd.sem_clear(r)

    nc.compile()
    return nc


def make_in_maps(chr_full: np.ndarray, ce: np.ndarray):
    ce_f32 = np.asarray(ce, dtype=np.float32)
    idx = np.asarray(chr_full).astype(np.int64) - 1
    maps = []
    for c in range(N_CORES):
        rows = ce_f32[idx[c * BPC : (c + 1) * BPC]]  # [64, 128]
        # Sample-interleaved duplicate: partition p holds sample p//2, so
        # partition p maps to DRAM outer indices (b=p//2, r=p%2).
        both = np.repeat(rows, 2, axis=0)  # [128, 128]
        pre = np.repeat(both[:, None, :], W0, axis=1)  # [128, W0, 128]
        maps.append({"pre": np.ascontiguousarray(pre)})
    return maps


def kernel(tensor=None, chr=None, ce=None, **_unused):
    chr_np = np.asarray(chr)
    ce_np = np.asarray(ce)
    nc = build_nc()
    res = run_bass_kernel_spmd(
        nc, make_in_maps(chr_np, ce_np), core_ids=list(range(N_CORES))
    )
    out = np.concatenate([r["out"] for r in res.results], axis=0)
    return out.astype(np.float32)
